# revision 9
# baseline (speedup 1.0000x reference)
"""Trainium2 Bass kernel for nn_Attention_4183298146960.

GQA causal attention layer: B=2, S=2048, HIDDEN=2048, 16 q heads / 4 kv heads,
head_dim=128, RoPE (interleaved pairs), causal softmax, output projection.

Sharding (8 cores, SPMD-uniform program):
  core c owns q heads {2c, 2c+1} and kv head c//2, for BOTH batches
  (tokens axis = [batch0 | batch1] = 4096).

  QKV: each core projects its 2 q heads (256 feats) plus its kv head.
  For batch 0 both k (feature-major + rope) and v (token-major, no rope) are
  computed locally -- no collective on the critical path while the multi-core
  rendezvous barrier (~50us) clears.  For batch 1 the kv work is SPLIT across
  the core pair: even cores compute k (rope), odd cores compute v (identity
  rope via per-core cos=1/sin=0 tables) from a per-core weight slot, and a
  2-rank AllGather exchanges the [128, 2048] panels (saves 1 GFLOP/core,
  fully hidden under the S2 interleave).  The exchanged v arrives
  feature-major and is transposed to token-major on the PE.

  Output projection is TOKEN-sharded: attention outputs are redistributed
  with one AllToAll per half-batch (4 total, 128-token shards) so W_o work
  unblocks progressively -- 8x less collective wire than AllGathering the
  full activation.  Each core computes the FULL 2048 output features for its
  4x128 tokens, with the attention panel as the STATIONARY matmul operand
  (activations = lhsT, W_o streams 512-wide), so W_o stays resident in SBUF
  and is loaded exactly once.

  The whole kernel is software-pipelined over the two batches:
    S1: QKV(b0)            S2: attn(b0) + A2As  interleaved with QKV(b1)
    S3: attn(b1) + A2As  interleaved with Wo(b0) then Wo(b1)   S4: drain
  Interleaving is done at matmul granularity (generators merged by a driver)
  so ScalarE exp / DVE softmax work overlaps dense PE phases and the PE never
  waits on the activation pipeline.

Layouts on device (partition dim first): feature-major qT/kT [head_dim, tok]
for scores; token-major v [tok, head_dim] for PV; scores computed transposed
[k, q] so softmax needs no max-subtraction and the denominator is a
ones-matrix matmul producing the broadcast denominator directly; probs stay
unnormalized until after PV.  The causal diagonal 512-block is computed at
128-granularity per head (narrowed matmuls) so only the [128,128] diagonal
sub-blocks need masking.
"""

import itertools

import numpy as np
import ml_dtypes

import concourse.bass as bass
import concourse.mybir as mybir
import concourse.tile as tile
from concourse import bacc
from concourse.bass_utils import run_bass_kernel_spmd

BF16 = ml_dtypes.bfloat16

HEADS = 16
KV_HEADS = 4
HIDDEN = 2048
HD = 128
S = 2048
B = 2
T = B * S                      # 4096 token axis (both batches)
HT = HIDDEN // 128             # 16 hidden tiles
SCALE = 1.0 / float(np.sqrt(HD))
RG8 = [[0, 1, 2, 3, 4, 5, 6, 7]]
RGPAIR = [[0, 1], [2, 3], [4, 5], [6, 7]]

_COMPILED = None


def _build():
    dt = mybir.dt
    f32 = dt.float32
    bf16 = dt.bfloat16
    nc = bacc.Bacc("TRN2", target_bir_lowering=False, debug=False, num_devices=8)

    xT = nc.dram_tensor("xT", [128, HT, T], bf16, kind="ExternalInput")
    # wqk feature slots: [q0, q1, k_perm, v, kv_half(per-core)]
    wqk = nc.dram_tensor("wqk", [128, HT, 640], bf16, kind="ExternalInput")
    ccq = nc.dram_tensor("ccq", [128, S], bf16, kind="ExternalInput")
    ssq = nc.dram_tensor("ssq", [128, S], bf16, kind="ExternalInput")
    cck = nc.dram_tensor("cck", [128, S], bf16, kind="ExternalInput")
    ssk = nc.dram_tensor("ssk", [128, S], bf16, kind="ExternalInput")
    msk = nc.dram_tensor("msk", [128, 128], bf16, kind="ExternalInput")
    ones128 = nc.dram_tensor("ones128", [128, 128], bf16, kind="ExternalInput")
    ident128 = nc.dram_tensor("ident128", [128, 128], bf16, kind="ExternalInput")
    wo = nc.dram_tensor("wo", [128, 16, 2048], bf16, kind="ExternalInput")
    outT = nc.dram_tensor("outT", [128, B, 2, 2048], f32, kind="ExternalOutput")

    mult = mybir.AluOpType.mult
    addop = mybir.AluOpType.add
    byp = mybir.AluOpType.bypass
    Exp = mybir.ActivationFunctionType.Exp

    with tile.TileContext(nc) as tc:
        with (
            tc.tile_pool(name="const", bufs=1) as constp,
            tc.tile_pool(name="dram", bufs=1, space="DRAM") as dram,
            tc.tile_pool(name="pp", bufs=4) as pp,
            tc.tile_pool(name="ap2", bufs=2) as ap2,
            tc.tile_pool(name="qps", bufs=1, space="PSUM") as qps,
            tc.tile_pool(name="sps", bufs=4, space="PSUM") as sps,
            tc.tile_pool(name="pvps", bufs=1, space="PSUM") as pvps,
        ):
            # ---- persistent SBUF ----
            qcat = constp.tile([128, 8, 2, 512], bf16)      # q feature-major
            kT = constp.tile([128, T], bf16)
            vsb = constp.tile([128, T], bf16)               # token-major v
            wqk_sb = constp.tile([128, HT, 640], bf16)
            ccq_sb = constp.tile([128, S], bf16)
            ssq_sb = constp.tile([128, S], bf16)
            cck_sb = constp.tile([128, S], bf16)
            ssk_sb = constp.tile([128, S], bf16)
            msk_sb = constp.tile([128, 128], bf16)
            ones_sb = constp.tile([128, 128], bf16)
            ident_sb = constp.tile([128, 128], bf16)

            # ---- DRAM scratch ----
            kvl = dram.tile([128, S], bf16, name="kvl")
            kvp = dram.tile([2, 128, S], bf16, name="kvp")
            a2i = [[dram.tile([8, 256, 128], bf16, name=f"a2i{b}{h}")
                    for h in range(2)] for b in range(B)]
            a2o = [[dram.tile([8, 256, 128], bf16, name=f"a2o{b}{h}")
                    for h in range(2)] for b in range(B)]

            xtiles = [None] * 4

            def gen_attn(b):
                """Attention for batch b.  Yields the number of filler quanta
                wanted at each point."""
                if b == 1:
                    # k/v panels from the pair exchange
                    nc.sync.dma_start(kT[:, S:2 * S], kvp[0])
                    vfeat = constp.tile([128, S], bf16, name="vfeat", bufs=1)
                    nc.sync.dma_start(vfeat[:], kvp[1])
                    yield 2
                    for tt in range(16):
                        trp = sps.tile([128, 128], bf16, name="sc")
                        nc.tensor.transpose(
                            trp[:], vfeat[:, tt * 128:(tt + 1) * 128], ident_sb[:]
                        )
                        nc.scalar.copy(
                            vsb[:, (16 + tt) * 128:(16 + tt + 1) * 128], trp[:]
                        )
                        if tt % 4 == 3:
                            yield 1
                for qt in (3, 2, 1, 0):
                    qtg = b * 4 + qt
                    pva = pvps.tile([128, 512], f32, name="pva")
                    pvb = pvps.tile([128, 512], f32, name="pvb")
                    pv = (pva, pvb)
                    acc = [
                        ap2.tile([128, 512], bf16, name=f"acc{h}") for h in (0, 1)
                    ]
                    nkt = 4 * qt
                    first = True
                    for kt in range(nkt):       # off-diagonal k tiles
                        ksl = slice(b * S + kt * 128, b * S + (kt + 1) * 128)
                        prs = []
                        for h in (0, 1):
                            sc = sps.tile([128, 512], f32, name="sc")
                            nc.tensor.matmul(
                                sc[:], lhsT=kT[:, ksl], rhs=qcat[:, qtg, h, :],
                                start=True, stop=True,
                            )
                            pr = pp.tile([128, 512], bf16, name="pr")
                            nc.scalar.activation(pr[:], sc[:], Exp, scale=SCALE)
                            prs.append(pr)
                        if kt % (2 + b) == 0:
                            yield 1
                        vsl = slice((b * 16 + kt) * 128, (b * 16 + kt + 1) * 128)
                        for h in (0, 1):
                            nc.tensor.matmul(
                                pv[h][:], lhsT=vsb[:, vsl], rhs=prs[h][:],
                                start=first, stop=False,
                            )
                        for h in (0, 1):
                            if first:
                                nc.vector.tensor_copy(acc[h][:], prs[h][:])
                            else:
                                nc.vector.tensor_add(acc[h][:], acc[h][:], prs[h][:])
                        first = False
                    for r in range(4):          # diagonal 512-block, narrowed
                        kt = nkt + r
                        w = 512 - 128 * r
                        ksl = slice(b * S + kt * 128, b * S + (kt + 1) * 128)
                        prs = []
                        for h in (0, 1):
                            sc = sps.tile([128, 512], f32, name="sc")
                            nc.tensor.matmul(
                                sc[:, 0:w], lhsT=kT[:, ksl],
                                rhs=qcat[:, qtg, h, 128 * r:512],
                                start=True, stop=True,
                            )
                            pr = pp.tile([128, 512], bf16, name="pr")
                            nc.scalar.activation(pr[:, 0:w], sc[:, 0:w], Exp, scale=SCALE)
                            nc.gpsimd.tensor_tensor(
                                pr[:, 0:128], pr[:, 0:128], msk_sb[:], mult
                            )
                            prs.append(pr)
                        yield 1
                        vsl = slice((b * 16 + kt) * 128, (b * 16 + kt + 1) * 128)
                        for h in (0, 1):
                            nc.tensor.matmul(
                                pv[h][:, 128 * r:512], lhsT=vsb[:, vsl],
                                rhs=prs[h][:, 0:w],
                                start=first, stop=(r == 3),
                            )
                        for h in (0, 1):
                            if first:
                                nc.vector.tensor_copy(acc[h][:], prs[h][:])
                            else:
                                nc.vector.tensor_add(
                                    acc[h][:, 128 * r:512], acc[h][:, 128 * r:512],
                                    prs[h][:, 0:w],
                                )
                        first = False
                    yield 1
                    for h in (0, 1):
                        den_ps = sps.tile([128, 512], f32, name="sc")
                        nc.tensor.matmul(
                            den_ps[:], lhsT=ones_sb[:], rhs=acc[h][:],
                            start=True, stop=True,
                        )
                        den_sb = ap2.tile([128, 512], f32, name="den")
                        nc.vector.reciprocal_approx_fast(den_sb[:], den_ps[:])
                        att = ap2.tile([128, 512], bf16, name="att")
                        nc.vector.tensor_tensor(att[:], pv[h][:], den_sb[:], mult)
                        for s2 in range(4):
                            nc.sync.dma_start(
                                a2i[b][qt // 2][(qt % 2) * 4 + s2,
                                                h * 128:(h + 1) * 128, :],
                                att[:, s2 * 128:(s2 + 1) * 128],
                            )
                    if qt == 2 or qt == 0:      # half-batch done -> redistribute
                        hh = qt // 2
                        nc.gpsimd.collective_compute(
                            "AllToAll", byp, replica_groups=RG8,
                            ins=[a2i[b][hh].opt()], outs=[a2o[b][hh].opt()],
                        )
                    yield 2

            with tc.tile_pool(name="xp", bufs=2) as xp, \
                 tc.tile_pool(name="rp", bufs=2) as rp:

                def emit_x_load(g, split_first=False):
                    xg = xp.tile([128, HT, 1024], bf16, name="xg")
                    if split_first:
                        nc.sync.dma_start(xg[:, 0:2, :], xT[:, 0:2, 0:1024])
                        for hq in range(4):
                            nc.sync.dma_start(
                                wqk_sb[:, hq * 4:(hq + 1) * 4, :],
                                wqk[:, hq * 4:(hq + 1) * 4, :],
                            )
                        nc.sync.dma_start(xg[:, 2:4, :], xT[:, 2:4, 0:1024])
                        for hq in range(1, 4):
                            nc.sync.dma_start(
                                xg[:, hq * 4:(hq + 1) * 4, :],
                                xT[:, hq * 4:(hq + 1) * 4, 0:1024],
                            )
                    else:
                        for hq in range(4):
                            nc.sync.dma_start(
                                xg[:, hq * 4:(hq + 1) * 4, :],
                                xT[:, hq * 4:(hq + 1) * 4, g * 1024:(g + 1) * 1024],
                            )
                    xtiles[g] = xg

                # startup DMAs: first x panel interleaved with wqk, then tables
                emit_x_load(0, split_first=True)
                nc.sync.dma_start(ccq_sb[:], ccq[:])
                nc.sync.dma_start(ssq_sb[:], ssq[:])
                nc.sync.dma_start(msk_sb[:], msk[:])
                nc.sync.dma_start(ones_sb[:], ones128[:])
                nc.sync.dma_start(ident_sb[:], ident128[:])
                emit_x_load(1)
                nc.sync.dma_start(cck_sb[:], cck[:])
                nc.sync.dma_start(ssk_sb[:], ssk[:])

                def do_ft(b, gi, ft):
                    """One [128-feature x 1024-token] projection + rope.
                    ft: 0/1 = q heads, 2 = k (batch0), 4 = kv half (batch1).
                    Batch 0 runs alone (S1) and borrows the idle 4-buffer
                    score pool so PSUM WAR never stalls the PE."""
                    g = 2 * b + gi
                    xg = xtiles[g]
                    fsl = slice(ft * 128, (ft + 1) * 128)
                    if b == 0:
                        qa = sps.tile([128, 512], f32, name="sc")
                        qb = sps.tile([128, 512], f32, name="sc")
                    else:
                        qa = qps.tile([128, 512], f32, name="qa")
                        qb = qps.tile([128, 512], f32, name="qb")
                    for ht in range(HT):
                        nc.tensor.matmul(
                            qa[:], lhsT=wqk_sb[:, ht, fsl], rhs=xg[:, ht, 0:512],
                            start=(ht == 0), stop=(ht == HT - 1),
                        )
                        nc.tensor.matmul(
                            qb[:], lhsT=wqk_sb[:, ht, fsl], rhs=xg[:, ht, 512:1024],
                            start=(ht == 0), stop=(ht == HT - 1),
                        )
                        if ht % 2 == 1:
                            yield
                    sbq = rp.tile([128, 1024], bf16, name="sbq")
                    nc.scalar.copy(sbq[:, 0:512], qa[:])
                    nc.vector.tensor_copy(sbq[:, 512:1024], qb[:])
                    tmp = rp.tile([128, 1024], bf16, name="tmp")
                    nc.gpsimd.dma_start(tmp[0:64, :], sbq[64:128, :])
                    nc.gpsimd.dma_start(tmp[64:128, :], sbq[0:64, :])
                    cc_t, ss_t = (cck_sb, ssk_sb) if ft == 4 else (ccq_sb, ssq_sb)
                    tsl = slice(gi * 1024, (gi + 1) * 1024)
                    qcc = rp.tile([128, 1024], bf16, name="qcc")
                    nc.vector.tensor_tensor(qcc[:], sbq[:], cc_t[:, tsl], mult)
                    qss = rp.tile([128, 1024], bf16, name="qss")
                    nc.vector.tensor_tensor(qss[:], tmp[:], ss_t[:, tsl], mult)
                    if ft == 2:      # batch-0 k: straight into kT
                        nc.vector.tensor_tensor(
                            kT[:, gi * 1024:(gi + 1) * 1024], qcc[:], qss[:], addop
                        )
                    elif ft == 4:    # batch-1 kv half: stage for pair exchange
                        kvs = rp.tile([128, 1024], bf16, name="kvs")
                        nc.vector.tensor_tensor(kvs[:], qcc[:], qss[:], addop)
                        nc.sync.dma_start(
                            kvl[:, gi * 1024:(gi + 1) * 1024], kvs[:]
                        )
                        if gi == 1:
                            nc.gpsimd.collective_compute(
                                "AllGather", byp, replica_groups=RGPAIR,
                                ins=[kvl.opt()], outs=[kvp.opt()],
                            )
                    else:
                        for s2 in (0, 1):
                            nc.vector.tensor_tensor(
                                qcat[:, g * 2 + s2, ft, :],
                                qcc[:, s2 * 512:(s2 + 1) * 512],
                                qss[:, s2 * 512:(s2 + 1) * 512],
                                addop,
                            )
                    yield

                def do_v(gi):
                    """Batch-0 v, computed token-major directly (no rope)."""
                    xg = xtiles[gi]
                    for st in range(8):
                        ps = sps.tile([128, 512], f32, name="sc")
                        for ht in range(HT):
                            nc.tensor.matmul(
                                ps[:, 0:128],
                                lhsT=xg[:, ht, st * 128:(st + 1) * 128],
                                rhs=wqk_sb[:, ht, 384:512],
                                start=(ht == 0), stop=(ht == HT - 1),
                            )
                        t128 = gi * 8 + st
                        nc.scalar.copy(
                            vsb[:, t128 * 128:(t128 + 1) * 128], ps[:, 0:128]
                        )
                        yield

                def gen_qkv0():
                    """Batch-0 QKV ordered so early work only needs the
                    x panel that has already arrived: k0, v0, k1, v1, q."""
                    for gi in (0, 1):
                        yield from do_ft(0, gi, 2)          # k
                        yield from do_v(gi)                 # v token-major
                    for gi in (0, 1):
                        if gi == 1:
                            emit_x_load(2)
                            emit_x_load(3)
                        for ft in (0, 1):
                            yield from do_ft(0, gi, ft)

                def gen_kv1():
                    """Batch-1 kv half -> pair exchange (run eagerly at S2
                    start so the AllGather fires as soon as the rendezvous
                    barrier clears)."""
                    for gi in (0, 1):
                        yield from do_ft(1, gi, 4)

                def gen_q1():
                    """Batch-1 q projections (S2 interleave filler)."""
                    for gi in (0, 1):
                        for ft in (0, 1):
                            yield from do_ft(1, gi, ft)

                def drive(lead, filler):
                    budget = 0
                    done = False
                    for req in lead:
                        budget += req
                        while budget > 0 and not done:
                            try:
                                next(filler)
                            except StopIteration:
                                done = True
                            budget -= 1
                    while not done:
                        try:
                            next(filler)
                        except StopIteration:
                            done = True

                # S1: QKV(b0), then batch-1 kv half (launches the exchange)
                for _ in gen_qkv0():
                    pass
                for _ in gen_kv1():
                    pass
                # S2: attn(b0) + A2As, filled with the batch-1 q projections
                drive(gen_attn(0), gen_q1())

            # x/rope pools released -> SBUF for the resident W_o panel
            with tc.tile_pool(name="wop", bufs=2) as wop:
                woB = wop.tile([128, HT, 2048], bf16, name="woB", bufs=1)

                def gen_wo(b):
                    """Output projection for this core's 4x128 tokens: the
                    attention panel is the stationary operand, W_o streams."""
                    if b == 0:
                        for dtt in range(HT):
                            nc.sync.dma_start(woB[:, dtt, :], wo[:, dtt, :])
                            if dtt % 4 == 3:
                                yield
                    for hh in (1, 0):
                        asb = wop.tile([128, 16, 128], bf16, name="asb")
                        for dtt in range(16):
                            nc.sync.dma_start(
                                asb[:, dtt, :],
                                a2o[b][hh][dtt // 2,
                                           (dtt % 2) * 128:(dtt % 2) * 128 + 128, :],
                            )
                        yield
                        for ofh in (0, 1):
                            qa = qps.tile([128, 512], f32, name="qa")
                            qb = qps.tile([128, 512], f32, name="qb")
                            for dtt in range(16):
                                o0 = ofh * 1024
                                nc.tensor.matmul(
                                    qa[:], lhsT=asb[:, dtt, :],
                                    rhs=woB[:, dtt, o0:o0 + 512],
                                    start=(dtt == 0), stop=(dtt == 15),
                                )
                                nc.tensor.matmul(
                                    qb[:], lhsT=asb[:, dtt, :],
                                    rhs=woB[:, dtt, o0 + 512:o0 + 1024],
                                    start=(dtt == 0), stop=(dtt == 15),
                                )
                                if dtt % 4 == 3:
                                    yield
                            for k2, ps in ((0, qa), (1, qb)):
                                osb = wop.tile([128, 512], f32, name="osb")
                                nc.vector.tensor_copy(osb[:], ps[:])
                                nc.sync.dma_start(
                                    outT[:, b, hh,
                                         ofh * 1024 + k2 * 512:
                                         ofh * 1024 + (k2 + 1) * 512],
                                    osb[:],
                                )
                            yield

                def delay_gen(n):
                    for _ in range(n):
                        yield

                def drive(lead, filler):
                    budget = 0
                    done = False
                    for req in lead:
                        budget += req
                        while budget > 0 and not done:
                            try:
                                next(filler)
                            except StopIteration:
                                done = True
                            budget -= 1
                    while not done:
                        try:
                            next(filler)
                        except StopIteration:
                            done = True

                # S3: attn(b1) + A2As, filled with Wo(b0) then Wo(b1)
                drive(
                    gen_attn(1),
                    itertools.chain(gen_wo(0), delay_gen(12), gen_wo(1)),
                )

    nc.compile()
    return nc


# host-side input prep ------------------------------------------------------

_PERM = np.concatenate([np.arange(0, HD, 2), np.arange(1, HD, 2)])


def _rope_tables():
    freq = 1.0 / (10000.0 ** (np.arange(0, HD, 2, dtype=np.float64) / HD))
    pos = np.arange(S, dtype=np.float64)
    ang = np.outer(pos, freq)                       # [S, 64]
    cos = np.cos(ang).T.astype(np.float32)          # [64, S]
    sin = np.sin(ang).T.astype(np.float32)
    ccq = np.concatenate([cos, cos], 0)             # [128, S]
    ssq = np.concatenate([-sin, sin], 0)
    return ccq.astype(BF16), ssq.astype(BF16)


def _prep_inputs(x, W_qkv, W_o):
    x = np.asarray(x, dtype=np.float32)
    W_qkv = np.asarray(W_qkv, dtype=np.float32)
    W_o = np.asarray(W_o, dtype=np.float32)

    xx = np.concatenate([x[0], x[1]], axis=0)       # [4096, 2048]
    xTd = np.ascontiguousarray(
        xx.T.reshape(HT, 128, T).transpose(1, 0, 2)
    ).astype(BF16)                                   # [128, HT, 4096]

    ccq, ssq = _rope_tables()
    cc_one = np.ones((128, S), dtype=np.float32).astype(BF16)
    ss_zero = np.zeros((128, S), dtype=np.float32).astype(BF16)

    ii = np.arange(128)[:, None]
    jj = np.arange(128)[None, :]
    mask = (jj >= ii).astype(np.float32).astype(BF16)
    ones128 = np.ones((128, 128), dtype=np.float32).astype(BF16)
    ident128 = np.eye(128, dtype=np.float32).astype(BF16)

    # wo[p, dtt, of] = W_o[of, dtt*128+p]
    woT = np.ascontiguousarray(
        W_o.T.reshape(16, 128, 2048).transpose(1, 0, 2)
    ).astype(BF16)                                   # [128, 16, 2048]

    in_maps = []
    for c in range(8):
        kh = c // 2
        qr = W_qkv[256 * c: 256 * (c + 1)]           # rows of q heads 2c,2c+1
        qr = qr.reshape(2, HD, HIDDEN)[:, _PERM, :].reshape(256, HIDDEN)
        kr = W_qkv[HIDDEN + 128 * kh: HIDDEN + 128 * (kh + 1)][_PERM, :]
        vr = W_qkv[HIDDEN + 512 + 128 * kh: HIDDEN + 512 + 128 * (kh + 1)]
        if c % 2 == 0:
            kvh = kr
            cck, ssk = ccq, ssq
        else:
            kvh = vr
            cck, ssk = cc_one, ss_zero
        wqkT = np.ascontiguousarray(
            np.concatenate([qr, kr, vr, kvh], 0)
            .T.reshape(HT, 128, 640).transpose(1, 0, 2)
        ).astype(BF16)                               # [128, HT, 640]
        in_maps.append({
            "xT": xTd, "wqk": wqkT, "wo": woT,
            "ccq": ccq, "ssq": ssq, "cck": cck, "ssk": ssk,
            "msk": mask, "ones128": ones128, "ident128": ident128,
        })
    return in_maps


def kernel(x, W_qkv, W_o):
    global _COMPILED
    if _COMPILED is None:
        _COMPILED = _build()
    nc = _COMPILED
    in_maps = _prep_inputs(x, W_qkv, W_o)
    res = run_bass_kernel_spmd(nc, in_maps, list(range(8)))
    out = np.empty((B, S, HIDDEN), dtype=np.float32)
    for c in range(8):
        oT = res.results[c]["outT"]                  # [128, B, 2, 2048]
        for b in range(B):
            for hh in range(2):
                out[b, hh * 1024 + c * 128: hh * 1024 + (c + 1) * 128, :] = (
                    oT[:, b, hh, :]
                )
    return out


# revision 10
# speedup vs baseline: 1.0779x; 1.0779x over previous
"""Trainium2 Bass kernel for nn_Attention_4183298146960.

GQA causal attention layer: B=2, S=2048, HIDDEN=2048, 16 q heads / 4 kv heads,
head_dim=128, RoPE (interleaved pairs), causal softmax, output projection.

Sharding (8 cores, SPMD-uniform program):
  core c owns q heads {2c, 2c+1} and kv head c//2, for BOTH batches
  (tokens axis = [batch0 | batch1] = 4096).

  QKV: each core projects its 2 q heads (256 feats) plus its kv head.
  For batch 0 both k (feature-major + rope) and v (token-major, no rope) are
  computed locally -- no collective on the critical path while the multi-core
  rendezvous barrier (~50us) clears.  For batch 1 the kv work is SPLIT across
  the core pair: even cores compute k (rope), odd cores compute v (identity
  rope via per-core cos=1/sin=0 tables) from a per-core weight slot, and a
  2-rank AllGather exchanges the [128, 2048] panels (saves 1 GFLOP/core,
  fully hidden under the S2 interleave).  The exchanged v arrives
  feature-major and is transposed to token-major on the PE.

  Output projection is TOKEN-sharded: attention outputs are redistributed
  with one AllToAll per half-batch (4 total, 128-token shards) so W_o work
  unblocks progressively -- 8x less collective wire than AllGathering the
  full activation.  Each core computes the FULL 2048 output features for its
  4x128 tokens, with the attention panel as the STATIONARY matmul operand
  (activations = lhsT, W_o streams 512-wide), so W_o stays resident in SBUF
  and is loaded exactly once.

  The whole kernel is software-pipelined over the two batches:
    S1: QKV(b0)            S2: attn(b0) + A2As  interleaved with QKV(b1)
    S3: attn(b1) + A2As  interleaved with Wo(b0) then Wo(b1)   S4: drain
  Interleaving is done at matmul granularity (generators merged by a driver)
  so ScalarE exp / DVE softmax work overlaps dense PE phases and the PE never
  waits on the activation pipeline.

Layouts on device (partition dim first): feature-major qT/kT [head_dim, tok]
for scores; token-major v [tok, head_dim] for PV; scores computed transposed
[k, q] so softmax needs no max-subtraction and the denominator is a
ones-matrix matmul producing the broadcast denominator directly; probs stay
unnormalized until after PV.  The causal diagonal 512-block is computed at
128-granularity per head (narrowed matmuls) so only the [128,128] diagonal
sub-blocks need masking.
"""

import itertools

import numpy as np
import ml_dtypes

import concourse.bass as bass
import concourse.mybir as mybir
import concourse.tile as tile
from concourse import bacc
from concourse.bass_utils import run_bass_kernel_spmd

BF16 = ml_dtypes.bfloat16

HEADS = 16
KV_HEADS = 4
HIDDEN = 2048
HD = 128
S = 2048
B = 2
T = B * S                      # 4096 token axis (both batches)
HT = HIDDEN // 128             # 16 hidden tiles
SCALE = 1.0 / float(np.sqrt(HD))
RG8 = [[0, 1, 2, 3, 4, 5, 6, 7]]
RGPAIR = [[0, 1], [2, 3], [4, 5], [6, 7]]

_COMPILED = None


def _build():
    dt = mybir.dt
    f32 = dt.float32
    bf16 = dt.bfloat16
    nc = bacc.Bacc("TRN2", target_bir_lowering=False, debug=False, num_devices=8)

    xT = nc.dram_tensor("xT", [128, HT, T], bf16, kind="ExternalInput")
    # wqk feature slots: [q0, q1, k_perm, v, kv_half(per-core)]
    wqk = nc.dram_tensor("wqk", [128, HT, 640], bf16, kind="ExternalInput")
    ccq = nc.dram_tensor("ccq", [128, S], bf16, kind="ExternalInput")
    ssq = nc.dram_tensor("ssq", [128, S], bf16, kind="ExternalInput")
    cck = nc.dram_tensor("cck", [128, S], bf16, kind="ExternalInput")
    ssk = nc.dram_tensor("ssk", [128, S], bf16, kind="ExternalInput")
    msk = nc.dram_tensor("msk", [128, 128], bf16, kind="ExternalInput")
    ones128 = nc.dram_tensor("ones128", [128, 128], bf16, kind="ExternalInput")
    ident128 = nc.dram_tensor("ident128", [128, 128], bf16, kind="ExternalInput")
    wo = nc.dram_tensor("wo", [128, 16, 2048], bf16, kind="ExternalInput")
    outT = nc.dram_tensor("outT", [128, B, 2, 2048], f32, kind="ExternalOutput")

    mult = mybir.AluOpType.mult
    addop = mybir.AluOpType.add
    byp = mybir.AluOpType.bypass
    Exp = mybir.ActivationFunctionType.Exp

    with tile.TileContext(nc) as tc:
        with (
            tc.tile_pool(name="const", bufs=1) as constp,
            tc.tile_pool(name="dram", bufs=1, space="DRAM") as dram,
            tc.tile_pool(name="pp", bufs=4) as pp,
            tc.tile_pool(name="ap2", bufs=2) as ap2,
            tc.tile_pool(name="qps", bufs=1, space="PSUM") as qps,
            tc.tile_pool(name="sps", bufs=4, space="PSUM") as sps,
            tc.tile_pool(name="pvps", bufs=1, space="PSUM") as pvps,
        ):
            # ---- persistent SBUF ----
            qcat = constp.tile([128, 8, 2, 512], bf16)      # q feature-major
            kT = constp.tile([128, T], bf16)
            vsb = constp.tile([128, T], bf16)               # token-major v
            wqk_sb = constp.tile([128, HT, 640], bf16)
            ccq_sb = constp.tile([128, S], bf16)
            ssq_sb = constp.tile([128, S], bf16)
            cck_sb = constp.tile([128, S], bf16)
            ssk_sb = constp.tile([128, S], bf16)
            msk_sb = constp.tile([128, 128], bf16)
            ones_sb = constp.tile([128, 128], bf16)
            ident_sb = constp.tile([128, 128], bf16)

            # ---- DRAM scratch ----
            kvl = dram.tile([128, S], bf16, name="kvl")
            kvp = dram.tile([2, 128, S], bf16, name="kvp")
            a2i = [[dram.tile([8, 256, 128], bf16, name=f"a2i{b}{h}")
                    for h in range(2)] for b in range(B)]
            a2o = [[dram.tile([8, 256, 128], bf16, name=f"a2o{b}{h}")
                    for h in range(2)] for b in range(B)]

            xtiles = [None] * 4

            def gen_attn(b):
                """Attention for batch b.  Yields the number of filler quanta
                wanted at each point."""
                if b == 1:
                    # k/v panels from the pair exchange
                    nc.sync.dma_start(kT[:, S:2 * S], kvp[0])
                    vfeat = constp.tile([128, S], bf16, name="vfeat", bufs=1)
                    nc.sync.dma_start(vfeat[:], kvp[1])
                    yield 2
                    for tt in range(16):
                        trp = sps.tile([128, 128], bf16, name="sc")
                        nc.tensor.transpose(
                            trp[:], vfeat[:, tt * 128:(tt + 1) * 128], ident_sb[:]
                        )
                        nc.scalar.copy(
                            vsb[:, (16 + tt) * 128:(16 + tt + 1) * 128], trp[:]
                        )
                        if tt % 4 == 3:
                            yield 1
                for qt in (3, 2, 1, 0):
                    qtg = b * 4 + qt
                    pva = pvps.tile([128, 512], f32, name="pva")
                    pvb = pvps.tile([128, 512], f32, name="pvb")
                    pv = (pva, pvb)
                    acc = [
                        ap2.tile([128, 512], bf16, name=f"acc{h}") for h in (0, 1)
                    ]
                    nkt = 4 * qt
                    first = True
                    for kt in range(nkt):       # off-diagonal k tiles
                        ksl = slice(b * S + kt * 128, b * S + (kt + 1) * 128)
                        prs = []
                        for h in (0, 1):
                            sc = sps.tile([128, 512], f32, name="sc")
                            nc.tensor.matmul(
                                sc[:], lhsT=kT[:, ksl], rhs=qcat[:, qtg, h, :],
                                start=True, stop=True,
                            )
                            pr = pp.tile([128, 512], bf16, name="pr")
                            nc.scalar.activation(pr[:], sc[:], Exp, scale=SCALE)
                            prs.append(pr)
                        if kt % (2 + b) == 0:
                            yield 1
                        vsl = slice((b * 16 + kt) * 128, (b * 16 + kt + 1) * 128)
                        for h in (0, 1):
                            nc.tensor.matmul(
                                pv[h][:], lhsT=vsb[:, vsl], rhs=prs[h][:],
                                start=first, stop=False,
                            )
                        for h in (0, 1):
                            if first:
                                nc.vector.tensor_copy(acc[h][:], prs[h][:])
                            else:
                                nc.vector.tensor_add(acc[h][:], acc[h][:], prs[h][:])
                        first = False
                    for r in range(4):          # diagonal 512-block, narrowed
                        kt = nkt + r
                        w = 512 - 128 * r
                        ksl = slice(b * S + kt * 128, b * S + (kt + 1) * 128)
                        prs = []
                        for h in (0, 1):
                            sc = sps.tile([128, 512], f32, name="sc")
                            nc.tensor.matmul(
                                sc[:, 0:w], lhsT=kT[:, ksl],
                                rhs=qcat[:, qtg, h, 128 * r:512],
                                start=True, stop=True,
                            )
                            pr = pp.tile([128, 512], bf16, name="pr")
                            nc.scalar.activation(pr[:, 0:w], sc[:, 0:w], Exp, scale=SCALE)
                            nc.vector.tensor_tensor(
                                pr[:, 0:128], pr[:, 0:128], msk_sb[:], mult
                            )
                            prs.append(pr)
                        yield 1
                        vsl = slice((b * 16 + kt) * 128, (b * 16 + kt + 1) * 128)
                        for h in (0, 1):
                            nc.tensor.matmul(
                                pv[h][:, 128 * r:512], lhsT=vsb[:, vsl],
                                rhs=prs[h][:, 0:w],
                                start=first, stop=(r == 3),
                            )
                        for h in (0, 1):
                            if first:
                                nc.vector.tensor_copy(acc[h][:], prs[h][:])
                            else:
                                nc.vector.tensor_add(
                                    acc[h][:, 128 * r:512], acc[h][:, 128 * r:512],
                                    prs[h][:, 0:w],
                                )
                        first = False
                    yield 1
                    for h in (0, 1):
                        den_ps = sps.tile([128, 512], f32, name="sc")
                        nc.tensor.matmul(
                            den_ps[:], lhsT=ones_sb[:], rhs=acc[h][:],
                            start=True, stop=True,
                        )
                        den_sb = ap2.tile([128, 512], f32, name="den")
                        nc.vector.reciprocal_approx_fast(den_sb[:], den_ps[:])
                        att = ap2.tile([128, 512], bf16, name="att")
                        nc.vector.tensor_tensor(att[:], pv[h][:], den_sb[:], mult)
                        for s2 in range(4):
                            nc.sync.dma_start(
                                a2i[b][qt // 2][(qt % 2) * 4 + s2,
                                                h * 128:(h + 1) * 128, :],
                                att[:, s2 * 128:(s2 + 1) * 128],
                            )
                    if qt == 2 or qt == 0:      # half-batch done -> redistribute
                        hh = qt // 2
                        nc.gpsimd.collective_compute(
                            "AllToAll", byp, replica_groups=RG8,
                            ins=[a2i[b][hh].opt()], outs=[a2o[b][hh].opt()],
                        )
                    yield 2

            with tc.tile_pool(name="xp", bufs=2) as xp, \
                 tc.tile_pool(name="rp", bufs=2) as rp:

                def emit_x_load(g, split_first=False):
                    xg = xp.tile([128, HT, 1024], bf16, name="xg")
                    if split_first:
                        nc.sync.dma_start(xg[:, 0:2, :], xT[:, 0:2, 0:1024])
                        for hq in range(4):
                            nc.sync.dma_start(
                                wqk_sb[:, hq * 4:(hq + 1) * 4, :],
                                wqk[:, hq * 4:(hq + 1) * 4, :],
                            )
                        nc.sync.dma_start(xg[:, 2:4, :], xT[:, 2:4, 0:1024])
                        for hq in range(1, 4):
                            nc.sync.dma_start(
                                xg[:, hq * 4:(hq + 1) * 4, :],
                                xT[:, hq * 4:(hq + 1) * 4, 0:1024],
                            )
                    else:
                        for hq in range(4):
                            nc.sync.dma_start(
                                xg[:, hq * 4:(hq + 1) * 4, :],
                                xT[:, hq * 4:(hq + 1) * 4, g * 1024:(g + 1) * 1024],
                            )
                    xtiles[g] = xg

                # startup DMAs: first x panel interleaved with wqk, then tables
                emit_x_load(0, split_first=True)
                nc.sync.dma_start(ccq_sb[:], ccq[:])
                nc.sync.dma_start(ssq_sb[:], ssq[:])
                nc.sync.dma_start(msk_sb[:], msk[:])
                nc.sync.dma_start(ones_sb[:], ones128[:])
                nc.sync.dma_start(ident_sb[:], ident128[:])
                emit_x_load(1)
                nc.sync.dma_start(cck_sb[:], cck[:])
                nc.sync.dma_start(ssk_sb[:], ssk[:])

                def do_ft(b, gi, ft):
                    """One [128-feature x 1024-token] projection + rope.
                    ft: 0/1 = q heads, 2 = k (batch0), 4 = kv half (batch1).
                    Batch 0 runs alone (S1) and borrows the idle 4-buffer
                    score pool so PSUM WAR never stalls the PE."""
                    g = 2 * b + gi
                    xg = xtiles[g]
                    fsl = slice(ft * 128, (ft + 1) * 128)
                    if b == 0:
                        qa = sps.tile([128, 512], f32, name="sc")
                        qb = sps.tile([128, 512], f32, name="sc")
                    else:
                        qa = qps.tile([128, 512], f32, name="qa")
                        qb = qps.tile([128, 512], f32, name="qb")
                    for ht in range(HT):
                        nc.tensor.matmul(
                            qa[:], lhsT=wqk_sb[:, ht, fsl], rhs=xg[:, ht, 0:512],
                            start=(ht == 0), stop=(ht == HT - 1),
                        )
                        nc.tensor.matmul(
                            qb[:], lhsT=wqk_sb[:, ht, fsl], rhs=xg[:, ht, 512:1024],
                            start=(ht == 0), stop=(ht == HT - 1),
                        )
                        if ht % 2 == 1:
                            yield
                    sbq = rp.tile([128, 1024], bf16, name="sbq")
                    nc.scalar.copy(sbq[:, 0:512], qa[:])
                    nc.scalar.copy(sbq[:, 512:1024], qb[:])
                    tmp = rp.tile([128, 1024], bf16, name="tmp")
                    nc.gpsimd.dma_start(tmp[0:64, :], sbq[64:128, :])
                    nc.gpsimd.dma_start(tmp[64:128, :], sbq[0:64, :])
                    cc_t, ss_t = (cck_sb, ssk_sb) if ft == 4 else (ccq_sb, ssq_sb)
                    tsl = slice(gi * 1024, (gi + 1) * 1024)
                    qcc = rp.tile([128, 1024], bf16, name="qcc")
                    nc.vector.tensor_tensor(qcc[:], sbq[:], cc_t[:, tsl], mult)
                    qss = rp.tile([128, 1024], bf16, name="qss")
                    nc.vector.tensor_tensor(qss[:], tmp[:], ss_t[:, tsl], mult)
                    if ft == 2:      # batch-0 k: straight into kT
                        nc.vector.tensor_tensor(
                            kT[:, gi * 1024:(gi + 1) * 1024], qcc[:], qss[:], addop
                        )
                    elif ft == 4:    # batch-1 kv half: stage for pair exchange
                        kvs = rp.tile([128, 1024], bf16, name="kvs")
                        nc.vector.tensor_tensor(kvs[:], qcc[:], qss[:], addop)
                        nc.sync.dma_start(
                            kvl[:, gi * 1024:(gi + 1) * 1024], kvs[:]
                        )
                        if gi == 1:
                            nc.gpsimd.collective_compute(
                                "AllGather", byp, replica_groups=RGPAIR,
                                ins=[kvl.opt()], outs=[kvp.opt()],
                            )
                    else:
                        for s2 in (0, 1):
                            nc.vector.tensor_tensor(
                                qcat[:, g * 2 + s2, ft, :],
                                qcc[:, s2 * 512:(s2 + 1) * 512],
                                qss[:, s2 * 512:(s2 + 1) * 512],
                                addop,
                            )
                    yield

                def do_v(gi):
                    """Batch-0 v, computed token-major directly (no rope)."""
                    xg = xtiles[gi]
                    for st in range(8):
                        ps = sps.tile([128, 512], f32, name="sc")
                        for ht in range(HT):
                            nc.tensor.matmul(
                                ps[:, 0:128],
                                lhsT=xg[:, ht, st * 128:(st + 1) * 128],
                                rhs=wqk_sb[:, ht, 384:512],
                                start=(ht == 0), stop=(ht == HT - 1),
                            )
                        t128 = gi * 8 + st
                        nc.scalar.copy(
                            vsb[:, t128 * 128:(t128 + 1) * 128], ps[:, 0:128]
                        )
                        yield

                def gen_qkv0():
                    """Batch-0 QKV ordered so early work only needs the
                    x panel that has already arrived: k0, v0, k1, v1, q."""
                    for gi in (0, 1):
                        yield from do_ft(0, gi, 2)          # k
                        yield from do_v(gi)                 # v token-major
                    for gi in (0, 1):
                        if gi == 1:
                            emit_x_load(2)
                            emit_x_load(3)
                        for ft in (0, 1):
                            yield from do_ft(0, gi, ft)

                def gen_kv1():
                    """Batch-1 kv half -> pair exchange (run eagerly at S2
                    start so the AllGather fires as soon as the rendezvous
                    barrier clears)."""
                    for gi in (0, 1):
                        yield from do_ft(1, gi, 4)

                def gen_q1():
                    """Batch-1 q projections (S2 interleave filler)."""
                    for gi in (0, 1):
                        for ft in (0, 1):
                            yield from do_ft(1, gi, ft)

                def drive(lead, filler):
                    budget = 0
                    done = False
                    for req in lead:
                        budget += req
                        while budget > 0 and not done:
                            try:
                                next(filler)
                            except StopIteration:
                                done = True
                            budget -= 1
                    while not done:
                        try:
                            next(filler)
                        except StopIteration:
                            done = True

                # S1: QKV(b0), then batch-1 kv half (launches the exchange)
                for _ in gen_qkv0():
                    pass
                for _ in gen_kv1():
                    pass
                # S2: attn(b0) + A2As, filled with the batch-1 q projections
                drive(gen_attn(0), gen_q1())

            # x/rope pools released -> SBUF for the resident W_o panel
            with tc.tile_pool(name="wop", bufs=2) as wop:
                woB = wop.tile([128, HT, 2048], bf16, name="woB", bufs=1)

                def gen_wo(b):
                    """Output projection for this core's 4x128 tokens: the
                    attention panel is the stationary operand, W_o streams
                    (paced through the first pass to keep DMA bandwidth free
                    for the concurrent AllToAlls)."""
                    if b == 0:
                        for dtt in range(4):
                            nc.sync.dma_start(woB[:, dtt, :], wo[:, dtt, :])
                        yield
                    for hh in (1, 0):
                        asb = wop.tile([128, 16, 128], bf16, name="asb")
                        for dtt in range(16):
                            nc.sync.dma_start(
                                asb[:, dtt, :],
                                a2o[b][hh][dtt // 2,
                                           (dtt % 2) * 128:(dtt % 2) * 128 + 128, :],
                            )
                        yield
                        for ofh in (0, 1):
                            qa = qps.tile([128, 512], f32, name="qa")
                            qb = qps.tile([128, 512], f32, name="qb")
                            for dtt in range(16):
                                if b == 0 and hh == 1 and ofh == 0 and dtt < 12:
                                    nc.sync.dma_start(
                                        woB[:, dtt + 4, :], wo[:, dtt + 4, :]
                                    )
                                o0 = ofh * 1024
                                nc.tensor.matmul(
                                    qa[:], lhsT=asb[:, dtt, :],
                                    rhs=woB[:, dtt, o0:o0 + 512],
                                    start=(dtt == 0), stop=(dtt == 15),
                                )
                                nc.tensor.matmul(
                                    qb[:], lhsT=asb[:, dtt, :],
                                    rhs=woB[:, dtt, o0 + 512:o0 + 1024],
                                    start=(dtt == 0), stop=(dtt == 15),
                                )
                                if dtt % 4 == 3:
                                    yield
                            for k2, ps in ((0, qa), (1, qb)):
                                osb = wop.tile([128, 512], f32, name="osb")
                                nc.scalar.copy(osb[:], ps[:])
                                nc.sync.dma_start(
                                    outT[:, b, hh,
                                         ofh * 1024 + k2 * 512:
                                         ofh * 1024 + (k2 + 1) * 512],
                                    osb[:],
                                )
                            yield

                def delay_gen(n):
                    for _ in range(n):
                        yield

                def drive(lead, filler):
                    budget = 0
                    done = False
                    for req in lead:
                        budget += req
                        while budget > 0 and not done:
                            try:
                                next(filler)
                            except StopIteration:
                                done = True
                            budget -= 1
                    while not done:
                        try:
                            next(filler)
                        except StopIteration:
                            done = True

                # S3: attn(b1) + A2As, filled with Wo(b0) then Wo(b1)
                drive(
                    gen_attn(1),
                    itertools.chain(gen_wo(0), delay_gen(12), gen_wo(1)),
                )

    nc.compile()
    return nc


# host-side input prep ------------------------------------------------------

_PERM = np.concatenate([np.arange(0, HD, 2), np.arange(1, HD, 2)])


def _rope_tables():
    freq = 1.0 / (10000.0 ** (np.arange(0, HD, 2, dtype=np.float64) / HD))
    pos = np.arange(S, dtype=np.float64)
    ang = np.outer(pos, freq)                       # [S, 64]
    cos = np.cos(ang).T.astype(np.float32)          # [64, S]
    sin = np.sin(ang).T.astype(np.float32)
    ccq = np.concatenate([cos, cos], 0)             # [128, S]
    ssq = np.concatenate([-sin, sin], 0)
    return ccq.astype(BF16), ssq.astype(BF16)


def _prep_inputs(x, W_qkv, W_o):
    x = np.asarray(x, dtype=np.float32)
    W_qkv = np.asarray(W_qkv, dtype=np.float32)
    W_o = np.asarray(W_o, dtype=np.float32)

    xx = np.concatenate([x[0], x[1]], axis=0)       # [4096, 2048]
    xTd = np.ascontiguousarray(
        xx.T.reshape(HT, 128, T).transpose(1, 0, 2)
    ).astype(BF16)                                   # [128, HT, 4096]

    ccq, ssq = _rope_tables()
    cc_one = np.ones((128, S), dtype=np.float32).astype(BF16)
    ss_zero = np.zeros((128, S), dtype=np.float32).astype(BF16)

    ii = np.arange(128)[:, None]
    jj = np.arange(128)[None, :]
    mask = (jj >= ii).astype(np.float32).astype(BF16)
    ones128 = np.ones((128, 128), dtype=np.float32).astype(BF16)
    ident128 = np.eye(128, dtype=np.float32).astype(BF16)

    # wo[p, dtt, of] = W_o[of, dtt*128+p]
    woT = np.ascontiguousarray(
        W_o.T.reshape(16, 128, 2048).transpose(1, 0, 2)
    ).astype(BF16)                                   # [128, 16, 2048]

    in_maps = []
    for c in range(8):
        kh = c // 2
        qr = W_qkv[256 * c: 256 * (c + 1)]           # rows of q heads 2c,2c+1
        qr = qr.reshape(2, HD, HIDDEN)[:, _PERM, :].reshape(256, HIDDEN)
        kr = W_qkv[HIDDEN + 128 * kh: HIDDEN + 128 * (kh + 1)][_PERM, :]
        vr = W_qkv[HIDDEN + 512 + 128 * kh: HIDDEN + 512 + 128 * (kh + 1)]
        if c % 2 == 0:
            kvh = kr
            cck, ssk = ccq, ssq
        else:
            kvh = vr
            cck, ssk = cc_one, ss_zero
        wqkT = np.ascontiguousarray(
            np.concatenate([qr, kr, vr, kvh], 0)
            .T.reshape(HT, 128, 640).transpose(1, 0, 2)
        ).astype(BF16)                               # [128, HT, 640]
        in_maps.append({
            "xT": xTd, "wqk": wqkT, "wo": woT,
            "ccq": ccq, "ssq": ssq, "cck": cck, "ssk": ssk,
            "msk": mask, "ones128": ones128, "ident128": ident128,
        })
    return in_maps


def kernel(x, W_qkv, W_o):
    global _COMPILED
    if _COMPILED is None:
        _COMPILED = _build()
    nc = _COMPILED
    in_maps = _prep_inputs(x, W_qkv, W_o)
    res = run_bass_kernel_spmd(nc, in_maps, list(range(8)))
    out = np.empty((B, S, HIDDEN), dtype=np.float32)
    for c in range(8):
        oT = res.results[c]["outT"]                  # [128, B, 2, 2048]
        for b in range(B):
            for hh in range(2):
                out[b, hh * 1024 + c * 128: hh * 1024 + (c + 1) * 128, :] = (
                    oT[:, b, hh, :]
                )
    return out
